# revision 1
# baseline (speedup 1.0000x reference)
"""8-layer GCN (PyG GCNConv semantics) on 8 Trainium2 NeuronCores.

Strategy
--------
Each conv is  h' = relu( dinv ⊙ (P @ (dinv ⊙ (h @ W))) + b )  where
P = A + I is an exact small-integer (0/1/2) matrix and dinv = deg^-1/2.
Using g = dinv ⊙ h, the recursion becomes

    x_l   = g_{l-1} @ W_l                  (dense transform, replicated)
    agg   = P @ x_l                        (row-sharded dense matmul, fp8 x fp16)
    g_l   = relu(dinv^2 ⊙ agg + dinv ⊙ b)  (per-partition scalars, node-major)

Rows of P (dst nodes) are sharded across 8 cores (1250 -> 1280 padded rows per
core, nodes padded 10000 -> 10240).  P is stored transposed + pre-swizzled in
fp8e4 (values exact) and kept resident in SBUF; x is fp16.  After each layer the
own shard is transposed to feature-major on the tensor engine (128x128 one-shot
via identity; the DVE "transpose" is only block-local 32x32), AllGathered in
feature-major layout in two node-halves (the first collective overlaps the
second half's aggregation), and the gathered matrix is read back with a single
plain DMA -- no DMA-transpose and no bounce copy (DMA-transposing the
collective's Shared output directly crashes the device; so does transposing
after an AllGather without an intervening copy).  Readout h8 @ Wr + br on DVE.

Numerics: fp16 transform + exact-fp8 P aggregation gives ~3.1e-3 relative
error vs the fp32 reference (validated by direct simulation and on hardware).
"""

import numpy as np
import ml_dtypes

import concourse.mybir as mybir
import concourse.tile as tile
from concourse import bacc
from concourse import bass_utils
from concourse.masks import make_identity

F16 = np.float16
FP8 = ml_dtypes.float8_e4m3

N_NODES = 10000
N_CORES = 8
DIMS = [128, 256, 256, 256, 128, 128, 64, 64, 32]
D_OUT = 1
SHARD = N_NODES // N_CORES          # 1250
SHARD_PAD = 1280                    # 10 j-tiles of 128
NPAD = N_CORES * SHARD_PAD          # 10240
ITILES = NPAD // 128                # 80
JTILES = SHARD_PAD // 128           # 10
FTILES = [max(1, d // 128) for d in DIMS[:-1]]  # contraction tiles per transform
# feature-major AG width per layer boundary (padded to a full partition tile)
AG_W = [max(DIMS[li + 1], 128) for li in range(7)]
# input stacking of ht_sb per layer: number of 128-row feature blocks per rank
STK = [1] + [AG_W[li] // 128 for li in range(7)]


def _build_bass(sim_mode=False, repeats=1, split_jt=6, psum_bufs=3):
    """sim_mode=True builds a single-core variant with the AllGather replaced
    by a local DMA copy so TimelineSim can run it (timing only)."""
    nc = bacc.Bacc(
        "TRN2",
        target_bir_lowering=False,
        debug=False,
        enable_asserts=False,
        num_devices=1 if sim_mode else N_CORES,
    )
    dt = mybir.dt

    pt_in = nc.dram_tensor("pt_in", [128, JTILES, ITILES, 128], dt.float8e4, kind="ExternalInput").ap()
    g0t_in = nc.dram_tensor("g0t_in", [128, NPAD], dt.float16, kind="ExternalInput").ap()
    dinv1_in = nc.dram_tensor("dinv1_in", [128, JTILES], dt.float32, kind="ExternalInput").ap()
    dinv2_in = nc.dram_tensor("dinv2_in", [128, JTILES], dt.float32, kind="ExternalInput").ap()
    w_ins = [
        nc.dram_tensor(f"w{i}_in", [128, FTILES[i], DIMS[i + 1]], dt.float16, kind="ExternalInput").ap()
        for i in range(8)
    ]
    bb_ins = [
        nc.dram_tensor(f"bb{i}_in", [128, DIMS[i + 1]], dt.float32, kind="ExternalInput").ap()
        for i in range(8)
    ]
    wr_in = nc.dram_tensor("wr_in", [128, DIMS[-1]], dt.float32, kind="ExternalInput").ap()
    br_in = nc.dram_tensor("br_in", [128, 1], dt.float32, kind="ExternalInput").ap()
    out_dram = nc.dram_tensor("out", [JTILES, 128, 1], dt.float32, kind="ExternalOutput").ap()

    # feature-major AllGather buffers, split in two node-halves per layer so
    # the first collective overlaps the second half's aggregation:
    # in [ag_w, 640], out [8*ag_w, 640]
    # uneven 7/3 split: the first (larger) gather hides under the remaining
    # aggregation; the exposed second gather is small
    HW_ = [(split_jt + 1) * 128, (JTILES - 1 - split_jt) * 128]
    ag_in = []
    ag_out = []
    for li in range(7):
        ag_in.append([
            nc.dram_tensor(f"agin{li}_{h}", [AG_W[li], HW_[h]], dt.float16, kind="Internal").ap()
            for h in range(2)
        ])
        ag_out.append([
            nc.dram_tensor(
                f"agout{li}_{h}", [N_CORES * AG_W[li], HW_[h]], dt.float16,
                kind="Internal", addr_space="Shared",
            ).ap()
            for h in range(2)
        ])

    with tile.TileContext(nc) as tc:
        with (
            tc.tile_pool(name="const", bufs=1) as const,
            tc.tile_pool(name="work", bufs=4) as work,
            tc.tile_pool(name="psum_t", bufs=psum_bufs, space="PSUM") as psum_t,
            tc.tile_pool(name="psum_a", bufs=psum_bufs, space="PSUM") as psum_a,
        ):
            pt_sb = const.tile([128, JTILES, ITILES, 128], dt.float8e4)
            for jt in range(JTILES):
                # per-j-tile slabs so the first aggregation isn't gated on the
                # full 13 MB adjacency load
                nc.sync.dma_start(pt_sb[:, jt, :, :], pt_in[:, jt, :, :])
            # gathered activations, feature-major, rank-stacked:
            # block r*stk+f holds rank r's features [f*128,(f+1)*128) x 1280 nodes
            ht_sb = const.tile([128, 16, SHARD_PAD], dt.float16)
            nc.sync.dma_start(ht_sb[:, 0:8, :], g0t_in[:])
            x_sb = const.tile([128, ITILES, 256], dt.float16)
            # own shard, feature-major, staging for the AllGather
            gt_sb = const.tile([128, 2, SHARD_PAD], dt.float16)
            dinv1_sb = const.tile([128, JTILES], dt.float32)
            nc.sync.dma_start(dinv1_sb[:], dinv1_in[:])
            dinv2_sb = const.tile([128, JTILES], dt.float32)
            nc.sync.dma_start(dinv2_sb[:], dinv2_in[:])
            w_sb = []
            bb_sb = []
            for li in range(8):
                w = const.tile([128, FTILES[li], DIMS[li + 1]], dt.float16, name=f"w{li}_sb")
                nc.sync.dma_start(w[:], w_ins[li][:])
                w_sb.append(w)
                bb = const.tile([128, DIMS[li + 1]], dt.float32, name=f"bb{li}_sb")
                nc.sync.dma_start(bb[:], bb_ins[li][:])
                bb_sb.append(bb)
            wr_sb = const.tile([128, DIMS[-1]], dt.float32)
            nc.sync.dma_start(wr_sb[:], wr_in[:])
            br_sb = const.tile([128, 1], dt.float32)
            nc.sync.dma_start(br_sb[:], br_in[:])
            ident = const.tile([128, 128], dt.float16)
            make_identity(nc, ident[:])

            for rep in range(repeats):
              if rep > 0:
                # timing-only repetition: reset the input activations
                nc.sync.dma_start(ht_sb[:, 0:8, :], g0t_in[:])
              for li in range(8):
                ft = FTILES[li]
                stk = STK[li]
                dout = DIMS[li + 1]

                # transform: x = g @ W  (node-major out), replicated on all
                # cores.  c-major order so the i-tiles covered by the first
                # half-AllGather can start while the second is in flight.
                for nci in range(0, ITILES, 2):
                    # paired i-tiles (r, r+1 at the same c): two matmul groups
                    # into one full PSUM bank, one strided copy for both --
                    # halves the copy instruction count on the DVE/ACT path
                    c, r = divmod(nci, N_CORES)
                    it = r * JTILES + c
                    pxs = psum_t.tile([128, 2, dout], dt.float32, tag="px", name="px")
                    for k in range(2):
                        for f in range(ft):
                            nc.tensor.matmul(
                                pxs[:, k, :],
                                ht_sb[:, (r + k) * stk + f, c * 128 : (c + 1) * 128],
                                w_sb[li][:, f, :],
                                start=(f == 0),
                                stop=(f == ft - 1),
                            )
                    dst = x_sb[:, it : it + JTILES + 1 : JTILES, :dout]
                    if (nci // 2) % 3 == 2:
                        # DVE alone bottlenecks the copy stream; offload 1/3 to ACT
                        nc.scalar.activation(
                            dst, pxs[:, :, :], mybir.ActivationFunctionType.Copy
                        )
                    else:
                        nc.vector.tensor_copy(dst, pxs[:, :, :])

                # aggregation: agg = P @ x  (own 1280 dst rows)
                for jt in range(JTILES):
                    pa = psum_a.tile([128, dout], dt.float32, tag="pa", name="pa")
                    for nci in range(ITILES):
                        # c-major: consume x tiles in the order they were produced
                        c, r = divmod(nci, N_CORES)
                        it = r * JTILES + c
                        nc.tensor.matmul(
                            pa[:],
                            pt_sb[:, jt, it, :],
                            x_sb[:, it, :dout],
                            start=(nci == 0),
                            stop=(nci == ITILES - 1),
                        )
                    if li < 7:
                        # g = relu(dinv^2*agg + dinv*b), fp16, then transpose the
                        # j-tile to feature-major staging for the AllGather
                        t = work.tile([128, dout], dt.float32, tag="ep_t", name="ep_t")
                        nc.vector.tensor_scalar(
                            t[:], pa[:], dinv2_sb[:, jt : jt + 1], None, op0=mybir.AluOpType.mult
                        )
                        b2 = work.tile([128, dout], dt.float32, tag="ep_b2", name="ep_b2")
                        nc.vector.tensor_scalar(
                            b2[:], bb_sb[li][:], dinv1_sb[:, jt : jt + 1], None, op0=mybir.AluOpType.mult
                        )
                        nc.vector.tensor_tensor(t[:], t[:], b2[:], mybir.AluOpType.add)
                        # transpose the j-tile to feature-major via the tensor
                        # engine (full 128x128 one-shot); pad narrow layers to a
                        # full 128-wide tile with zero columns.
                        gw = max(dout, 128)
                        g = work.tile([128, gw], dt.float16, tag="ep_g", name="ep_g")
                        if dout < 128:
                            nc.vector.memset(g[:, dout:], 0.0)
                        nc.scalar.activation(g[:, :dout], t[:], mybir.ActivationFunctionType.Relu)
                        for f in range(gw // 128):
                            ptr = psum_t.tile([128, 128], dt.float16, tag="ptr", name="ptr", bufs=2)
                            nc.tensor.transpose(
                                ptr[:], g[:, f * 128 : (f + 1) * 128], ident[:]
                            )
                            nc.vector.tensor_copy(
                                gt_sb[:, f, jt * 128 : (jt + 1) * 128], ptr[:]
                            )
                    else:
                        # h8 = relu(dinv*agg + b); readout r = h8 @ Wr + br on DVE
                        t = work.tile([128, dout], dt.float32, tag="ep_t", name="ep_t")
                        nc.vector.tensor_scalar(
                            t[:], pa[:], dinv1_sb[:, jt : jt + 1], None, op0=mybir.AluOpType.mult
                        )
                        nc.vector.tensor_tensor(t[:], t[:], bb_sb[li][:], mybir.AluOpType.add)
                        h8 = work.tile([128, dout], dt.float32, tag="ep_h8", name="ep_h8")
                        nc.scalar.activation(h8[:], t[:], mybir.ActivationFunctionType.Relu)
                        prod = work.tile([128, dout], dt.float32, tag="ep_pr", name="ep_pr")
                        nc.vector.tensor_tensor(prod[:], h8[:], wr_sb[:], mybir.AluOpType.mult)
                        red = work.tile([128, 1], dt.float32, tag="ep_red", name="ep_red")
                        nc.vector.reduce_sum(red[:], prod[:], axis=mybir.AxisListType.X)
                        nc.vector.tensor_scalar(
                            red[:], red[:], br_sb[:, 0:1], None, op0=mybir.AluOpType.add
                        )
                        nc.sync.dma_start(out_dram[jt, :, :], red[:])

                    if li < 7 and jt in (split_jt, 9):
                        # half-shard AllGather: emit as soon as this node-half's
                        # epilogues are staged so it overlaps the rest of the
                        # aggregation (and the next transform's early columns)
                        h = 0 if jt == split_jt else 1
                        nblk = AG_W[li] // 128
                        cols = slice(0, HW_[0]) if h == 0 else slice(HW_[0], SHARD_PAD)
                        for f in range(nblk):
                            nc.sync.dma_start(
                                ag_in[li][h][f * 128 : (f + 1) * 128, :],
                                gt_sb[:, f, cols],
                            )
                        if sim_mode:
                            nc.sync.dma_start(
                                ag_out[li][h][0 : AG_W[li], :], ag_in[li][h][:]
                            )
                        else:
                            nc.gpsimd.collective_compute(
                                "AllGather",
                                mybir.AluOpType.bypass,
                                replica_groups=[list(range(N_CORES))],
                                ins=[ag_in[li][h][:]],
                                outs=[ag_out[li][h][:]],
                            )
                        # plain (non-transposing) read from the Shared output is
                        # safe; block b=r*nblk+f lands in ht_sb slot b.
                        nc.sync.dma_start(
                            ht_sb[:, 0 : 8 * nblk, cols],
                            ag_out[li][h].rearrange("(b p) n -> p b n", p=128),
                        )

    nc.compile()
    return nc


_NC_CACHE = None


def _get_nc():
    global _NC_CACHE
    if _NC_CACHE is None:
        _NC_CACHE = _build_bass()
    return _NC_CACHE


def _pad_index(g):
    """global node id -> padded id (each core's 1250 rows padded to 1280)."""
    k = g // SHARD
    return k * SHARD_PAD + (g - k * SHARD)


def _prepare_inputs(inputs):
    x = np.asarray(inputs["x"], np.float32)
    ei = np.asarray(inputs["edge_index"])
    src, dst = ei[0].astype(np.int64), ei[1].astype(np.int64)

    deg = np.zeros(N_NODES, np.float32)
    np.add.at(deg, dst, 1.0)
    deg += 1.0  # self loop
    dinv = 1.0 / np.sqrt(deg)

    psrc = _pad_index(src)
    pdst = _pad_index(dst)
    pself = _pad_index(np.arange(N_NODES, dtype=np.int64))

    P = np.zeros((NPAD, NPAD), np.float32)
    np.add.at(P, (pdst, psrc), 1.0)
    P[pself, pself] += 1.0
    assert P.max() <= 15, "fp8e4 exactness bound exceeded"

    dinv_pad = np.zeros(NPAD, np.float32)
    dinv_pad[pself] = dinv

    # initial g0 = dinv * x, feature-major, padded
    g0 = dinv[:, None] * x  # [N, 128]
    g0t = np.zeros((128, NPAD), np.float32)
    g0t[:, pself] = g0.T
    g0t = g0t.astype(F16)

    # weights: [128, ftiles, dout] with zero padding on the contraction dim
    w_np = []
    for li in range(8):
        W = np.asarray(inputs[f"W{li}"], np.float32)
        ft = FTILES[li]
        Wp = np.zeros((ft * 128, DIMS[li + 1]), np.float32)
        Wp[: W.shape[0]] = W
        w_np.append(np.ascontiguousarray(Wp.reshape(ft, 128, DIMS[li + 1]).transpose(1, 0, 2)).astype(F16))
    bb_np = [
        np.broadcast_to(np.asarray(inputs[f"b{li}"], np.float32), (128, DIMS[li + 1])).copy()
        for li in range(8)
    ]
    wr = np.asarray(inputs["Wr"], np.float32)  # [32, 1]
    wr_np = np.broadcast_to(wr[:, 0], (128, DIMS[-1])).copy()
    br_np = np.full((128, 1), np.asarray(inputs["br"], np.float32).reshape(()), np.float32)

    in_maps = []
    for k in range(N_CORES):
        rows = slice(k * SHARD_PAD, (k + 1) * SHARD_PAD)
        # PT swizzle: pt[p, jt, it, c] = P[k*1280 + jt*128 + c, it*128 + p]
        S = P[rows].reshape(JTILES, 128, ITILES, 128)  # [jt, c, it, p]
        pt = np.ascontiguousarray(S.transpose(3, 0, 2, 1)).astype(FP8)
        dj = dinv_pad[rows].reshape(JTILES, 128).T.copy()  # [128, JTILES]
        m = {
            "pt_in": pt,
            "g0t_in": g0t,
            "dinv1_in": dj,
            "dinv2_in": (dj * dj).astype(np.float32),
            "wr_in": wr_np,
            "br_in": br_np,
        }
        for li in range(8):
            m[f"w{li}_in"] = w_np[li]
            m[f"bb{li}_in"] = bb_np[li]
        in_maps.append(m)
    return in_maps


def kernel(**inputs):
    nc = _get_nc()
    in_maps = _prepare_inputs(inputs)
    res = bass_utils.run_bass_kernel_spmd(nc, in_maps, core_ids=list(range(N_CORES)))
    out = np.empty((N_NODES, D_OUT), np.float32)
    for k in range(N_CORES):
        shard = res.results[k]["out"].reshape(SHARD_PAD, D_OUT)
        out[k * SHARD : (k + 1) * SHARD] = shard[:SHARD]
    return out



# revision 6
# speedup vs baseline: 93.6106x; 93.6106x over previous
"""8-layer GCN (PyG GCNConv semantics) on 8 Trainium2 NeuronCores.

Strategy (v2)
-------------
Each conv is  h' = relu( dinv ⊙ (P @ (dinv ⊙ (h @ W))) + b )  where
P = A + I is an exact small-integer matrix and dinv = deg^-1/2.  With
g = dinv ⊙ h the recursion is

    x_l   = g_{l-1} @ (W_l s_l)            (dense transform, node-SHARDED)
    agg   = P @ x_l                        (row-sharded, fp8 x fp8 DoubleRow)
    g_l   = relu(dinv^2/s_l ⊙ agg + dinv ⊙ b)

v2 differences from the replicated-transform baseline:
  * The transform is computed only for the core's own 1280 rows (8x less
    tensor-engine work); the *transformed* x_l is AllGathered node-major, so
    the gather path has no transposes at all (plain DMAs both directions).
  * x is quantized to fp8e4m3 with a per-layer power-of-two scale s_l
    calibrated on the host (folded into W_l and the epilogue constants, so
    it costs nothing).  fp8 halves the collective bytes everywhere and
    enables MatmulPerfMode.DoubleRow for the aggregation: two 128-src
    k-tiles per instruction at 0.5 cycles/row (2x) on the dout>=128 layers.
  * Layer 0's transform stays replicated (g0 is an input, so no collective
    is needed before the first aggregation).

The per-layer x_{l+1} own-shard is staged node-major in SBUF, DMAed to DRAM
in two node-halves (c-tiles 0..split and split+1..9), AllGathered, and read
back with one plain strided DMA per half into the x slot buffer (double
buffered, so the readback never waits on the previous layer's aggregation).
Numerics: ~5e-3 relative error vs the fp32 reference (validated in numpy
and on hardware).
"""

import numpy as np
import ml_dtypes

import concourse.mybir as mybir
import concourse.tile as tile
from concourse import bacc
from concourse import bass_utils
from concourse.masks import make_identity

F16 = np.float16
FP8 = ml_dtypes.float8_e4m3

N_NODES = 10000
N_CORES = 8
DIMS = [128, 256, 256, 256, 128, 128, 64, 64, 32]
D_OUT = 1
SHARD = N_NODES // N_CORES          # 1250
SHARD_PAD = 1280                    # 10 c-tiles of 128
NPAD = N_CORES * SHARD_PAD          # 10240
WTILES = NPAD // 128                # 80 (flat (r, c) tile index w = r*10 + c)
JTILES = SHARD_PAD // 128           # 10
FTILES = [max(1, d // 128) for d in DIMS[:-1]]  # contraction tiles per transform
# layers whose aggregation uses fp8 DoubleRow (2 k-tiles / instruction)
DR_LAYERS = (0, 1, 2, 3, 4)


def _build_bass(sim_mode=False, repeats=1, split_jt=7, psum_bufs=3,
                dr_layers=DR_LAYERS):
    """sim_mode=True builds a single-core variant with the AllGather replaced
    by a local DMA copy so TimelineSim can run it (timing only).
    split_jt must be odd so DoubleRow pairs never straddle the AG halves."""
    assert split_jt % 2 == 1
    nc = bacc.Bacc(
        "TRN2",
        target_bir_lowering=False,
        debug=False,
        enable_asserts=False,
        num_devices=1 if sim_mode else N_CORES,
    )
    dt = mybir.dt

    pt_in = nc.dram_tensor("pt_in", [128, JTILES, WTILES, 128], dt.float8e4, kind="ExternalInput").ap()
    g0t_in = nc.dram_tensor("g0t_in", [128, NPAD], dt.float16, kind="ExternalInput").ap()
    dinv1_in = nc.dram_tensor("dinv1_in", [128, JTILES], dt.float32, kind="ExternalInput").ap()
    dinv2s_in = nc.dram_tensor("dinv2s_in", [128, 7, JTILES], dt.float32, kind="ExternalInput").ap()
    dinv17_in = nc.dram_tensor("dinv17_in", [128, JTILES], dt.float32, kind="ExternalInput").ap()
    w_ins = [
        nc.dram_tensor(f"w{i}_in", [128, FTILES[i], DIMS[i + 1]], dt.float16, kind="ExternalInput").ap()
        for i in range(8)
    ]
    bb_ins = [
        nc.dram_tensor(f"bb{i}_in", [128, DIMS[i + 1]], dt.float32, kind="ExternalInput").ap()
        for i in range(8)
    ]
    wr_in = nc.dram_tensor("wr_in", [128, DIMS[-1]], dt.float32, kind="ExternalInput").ap()
    br_in = nc.dram_tensor("br_in", [128, 1], dt.float32, kind="ExternalInput").ap()
    out_dram = nc.dram_tensor("out", [JTILES, 128, 1], dt.float32, kind="ExternalOutput").ap()

    # node-major AllGather buffers for x_l (l = 1..7), two node-halves each
    HROWS = [(split_jt + 1) * 128, (JTILES - 1 - split_jt) * 128]
    agx_in = [None] * 8
    agx_out = [None] * 8
    for l in range(1, 8):
        d = DIMS[l + 1]
        agx_in[l] = [
            nc.dram_tensor(f"agxin{l}_{h}", [HROWS[h], d], dt.float8e4, kind="Internal").ap()
            for h in range(2)
        ]
        agx_out[l] = [
            nc.dram_tensor(
                f"agxout{l}_{h}", [N_CORES * HROWS[h], d], dt.float8e4,
                kind="Internal", addr_space="Shared",
            ).ap()
            for h in range(2)
        ]

    with tile.TileContext(nc) as tc:
        with (
            tc.tile_pool(name="const", bufs=1) as const,
            tc.tile_pool(name="work", bufs=4) as work,
            tc.tile_pool(name="psum_t", bufs=psum_bufs, space="PSUM") as psum_t,
            tc.tile_pool(name="psum_a", bufs=psum_bufs, space="PSUM") as psum_a,
        ):
            pt_sb = const.tile([128, JTILES, WTILES, 128], dt.float8e4)
            for jt in range(JTILES):
                nc.sync.dma_start(pt_sb[:, jt, :, :], pt_in[:, jt, :, :])
            g0t_sb = const.tile([128, NPAD], dt.float16)
            nc.sync.dma_start(g0t_sb[:], g0t_in[:])
            # x slot buffers, double buffered by layer parity: slot (r, c)
            # holds rank r's node c-tile, fp8, width padded to 256
            x8 = [
                const.tile([128, N_CORES, JTILES, 256], dt.float8e4, name=f"x8_{p}")
                for p in range(2)
            ]
            # own g feature-major (for the next transform), double buffered
            gt = [
                const.tile([128, 2, SHARD_PAD], dt.float16, name=f"gt_{p}")
                for p in range(2)
            ]
            # own x_{l+1} staging before the AllGather DMA, double buffered
            xo = [
                const.tile([128, JTILES, 256], dt.float8e4, name=f"xo_{p}")
                for p in range(2)
            ]
            dinv1_sb = const.tile([128, JTILES], dt.float32)
            nc.sync.dma_start(dinv1_sb[:], dinv1_in[:])
            dinv2s_sb = const.tile([128, 7, JTILES], dt.float32)
            nc.sync.dma_start(dinv2s_sb[:], dinv2s_in[:])
            dinv17_sb = const.tile([128, JTILES], dt.float32)
            nc.sync.dma_start(dinv17_sb[:], dinv17_in[:])
            w_sb = []
            bb_sb = []
            for li in range(8):
                w = const.tile([128, FTILES[li], DIMS[li + 1]], dt.float16, name=f"w{li}_sb")
                nc.sync.dma_start(w[:], w_ins[li][:])
                w_sb.append(w)
                bb = const.tile([128, DIMS[li + 1]], dt.float32, name=f"bb{li}_sb")
                nc.sync.dma_start(bb[:], bb_ins[li][:])
                bb_sb.append(bb)
            wr_sb = const.tile([128, DIMS[-1]], dt.float32)
            nc.sync.dma_start(wr_sb[:], wr_in[:])
            br_sb = const.tile([128, 1], dt.float32)
            nc.sync.dma_start(br_sb[:], br_in[:])
            ident = const.tile([128, 128], dt.float16)
            make_identity(nc, ident[:])

            for rep in range(repeats):
              for li in range(8):
                d = DIMS[li + 1]
                xi = x8[li % 2]           # this layer's x (all nodes)
                xn = x8[(li + 1) % 2]     # next layer's x (AllGather readback)
                gl = gt[li % 2]
                xol = xo[(li + 1) % 2]
                dr = li in dr_layers

                if li == 0:
                    # replicated transform x0 = g0 @ W0 for all 80 node tiles
                    # (g0 is an input, so no collective before the first agg)
                    for w in range(0, WTILES, 2):
                        pxs = psum_t.tile([128, 2, d], dt.float32, tag="px", name="px", bufs=2)
                        for k in range(2):
                            nc.tensor.matmul(
                                pxs[:, k, :],
                                g0t_sb[:, (w + k) * 128 : (w + k + 1) * 128],
                                w_sb[0][:, 0, :],
                                start=True,
                                stop=True,
                            )
                        r0, c0 = divmod(w, JTILES)
                        dst = xi[:, r0, c0 : c0 + 2, :d]
                        if (w // 2) % 5 < 3:
                            nc.vector.tensor_copy(dst, pxs[:, :, :])
                        else:
                            nc.scalar.activation(
                                dst, pxs[:, :, :], mybir.ActivationFunctionType.Copy
                            )

                for jt in range(JTILES):
                    # aggregation: agg = P @ x over all 80 src tiles; consume
                    # the first AG half (c <= split_jt) before the second
                    pa = psum_a.tile([128, d], dt.float32, tag="pa", name="pa", bufs=2)
                    if dr:
                        seq = [(r, c) for r in range(N_CORES) for c in range(0, split_jt + 1, 2)]
                        seq += [(r, c) for r in range(N_CORES) for c in range(split_jt + 1, JTILES, 2)]
                        for i, (r, c) in enumerate(seq):
                            w = r * JTILES + c
                            nc.tensor.matmul(
                                pa[:],
                                pt_sb[:, jt, w : w + 2, :],
                                xi[:, r, c : c + 2, :d],
                                start=(i == 0),
                                stop=(i == len(seq) - 1),
                                perf_mode=mybir.MatmulPerfMode.DoubleRow,
                            )
                    else:
                        seq = [(r, c) for r in range(N_CORES) for c in range(0, split_jt + 1)]
                        seq += [(r, c) for r in range(N_CORES) for c in range(split_jt + 1, JTILES)]
                        for i, (r, c) in enumerate(seq):
                            w = r * JTILES + c
                            nc.tensor.matmul(
                                pa[:],
                                pt_sb[:, jt, w, :],
                                xi[:, r, c, :d],
                                start=(i == 0),
                                stop=(i == len(seq) - 1),
                            )

                    if li < 7:
                        # g = relu(dinv^2/s ⊙ agg + dinv ⊙ b) in fp16,
                        # transposed to feature-major for the own transform
                        t = work.tile([128, d], dt.float32, tag="ep_t", name="ep_t")
                        nc.vector.tensor_scalar(
                            t[:], pa[:], dinv2s_sb[:, li, jt : jt + 1], None, op0=mybir.AluOpType.mult
                        )
                        b2 = work.tile([128, d], dt.float32, tag="ep_b2", name="ep_b2")
                        nc.vector.tensor_scalar(
                            b2[:], bb_sb[li][:], dinv1_sb[:, jt : jt + 1], None, op0=mybir.AluOpType.mult
                        )
                        nc.vector.tensor_tensor(t[:], t[:], b2[:], mybir.AluOpType.add)
                        g = work.tile([128, d], dt.float16, tag="ep_g", name="ep_g")
                        nc.scalar.activation(g[:], t[:], mybir.ActivationFunctionType.Relu)
                        for f in range((d + 127) // 128):
                            fw = min(128, d - f * 128)
                            ptr = psum_t.tile([128, 128], dt.float16, tag="ptr", name="ptr", bufs=2)
                            nc.tensor.transpose(
                                ptr[:fw, :], g[:, f * 128 : f * 128 + fw], ident[:]
                            )
                            nc.vector.tensor_copy(
                                gl[:fw, f, jt * 128 : (jt + 1) * 128], ptr[:fw, :]
                            )
                        # transform own c-tile for the NEXT layer:
                        # x_{l+1}[own jt] = g_l[own jt] @ W_{l+1}
                        d2 = DIMS[li + 2]
                        ft2 = FTILES[li + 1]
                        pxo = psum_t.tile([128, d2], dt.float32, tag="pxo", name="pxo", bufs=2)
                        for f in range(ft2):
                            nc.tensor.matmul(
                                pxo[:],
                                gl[:, f, jt * 128 : (jt + 1) * 128],
                                w_sb[li + 1][:, f, :],
                                start=(f == 0),
                                stop=(f == ft2 - 1),
                            )
                        if jt % 2 == 0:
                            nc.vector.tensor_copy(xol[:, jt, :d2], pxo[:])
                        else:
                            nc.scalar.activation(
                                xol[:, jt, :d2], pxo[:], mybir.ActivationFunctionType.Copy
                            )
                    else:
                        # h8 = relu(dinv/s ⊙ agg + b); readout h8 @ Wr + br
                        t = work.tile([128, d], dt.float32, tag="ep_t", name="ep_t")
                        nc.vector.tensor_scalar(
                            t[:], pa[:], dinv17_sb[:, jt : jt + 1], None, op0=mybir.AluOpType.mult
                        )
                        nc.vector.tensor_tensor(t[:], t[:], bb_sb[li][:], mybir.AluOpType.add)
                        h8 = work.tile([128, d], dt.float32, tag="ep_h8", name="ep_h8")
                        nc.scalar.activation(h8[:], t[:], mybir.ActivationFunctionType.Relu)
                        prod = work.tile([128, d], dt.float32, tag="ep_pr", name="ep_pr")
                        nc.vector.tensor_tensor(prod[:], h8[:], wr_sb[:], mybir.AluOpType.mult)
                        red = work.tile([128, 1], dt.float32, tag="ep_red", name="ep_red")
                        nc.vector.reduce_sum(red[:], prod[:], axis=mybir.AxisListType.X)
                        nc.vector.tensor_scalar(
                            red[:], red[:], br_sb[:, 0:1], None, op0=mybir.AluOpType.add
                        )
                        nc.sync.dma_start(out_dram[jt, :, :], red[:])

                    if li < 7 and jt in (split_jt, JTILES - 1):
                        # ship this node-half of own x_{l+1}: SBUF -> DRAM,
                        # AllGather, plain strided readback into the other
                        # x slot buffer
                        h = 0 if jt == split_jt else 1
                        c0 = 0 if h == 0 else split_jt + 1
                        ncr = (split_jt + 1) if h == 0 else (JTILES - 1 - split_jt)
                        d2 = DIMS[li + 2]
                        nc.sync.dma_start(
                            agx_in[li + 1][h].rearrange("(c p) d -> p c d", p=128),
                            xol[:, c0 : c0 + ncr, :d2],
                        )
                        if sim_mode:
                            nc.sync.dma_start(
                                agx_out[li + 1][h][0 : HROWS[h], :], agx_in[li + 1][h][:]
                            )
                        else:
                            nc.gpsimd.collective_compute(
                                "AllGather",
                                mybir.AluOpType.bypass,
                                replica_groups=[list(range(N_CORES))],
                                ins=[agx_in[li + 1][h][:]],
                                outs=[agx_out[li + 1][h][:]],
                            )
                        for r in range(N_CORES):
                            nc.sync.dma_start(
                                xn[:, r, c0 : c0 + ncr, :d2],
                                agx_out[li + 1][h][
                                    r * HROWS[h] : r * HROWS[h] + ncr * 128, :
                                ].rearrange("(c p) d -> p c d", p=128),
                            )

    nc.compile()
    return nc


_NC_CACHE = None


def _get_nc():
    global _NC_CACHE
    if _NC_CACHE is None:
        _NC_CACHE = _build_bass()
    return _NC_CACHE


def _pad_index(g):
    """global node id -> padded id (each core's 1250 rows padded to 1280)."""
    k = g // SHARD
    return k * SHARD_PAD + (g - k * SHARD)


def _prepare_inputs(inputs):
    x = np.asarray(inputs["x"], np.float32)
    ei = np.asarray(inputs["edge_index"])
    src, dst = ei[0].astype(np.int64), ei[1].astype(np.int64)

    deg = np.zeros(N_NODES, np.float32)
    np.add.at(deg, dst, 1.0)
    deg += 1.0  # self loop
    dinv = 1.0 / np.sqrt(deg)

    psrc = _pad_index(src)
    pdst = _pad_index(dst)
    pself = _pad_index(np.arange(N_NODES, dtype=np.int64))

    P = np.zeros((NPAD, NPAD), np.float32)
    np.add.at(P, (pdst, psrc), 1.0)
    P[pself, pself] += 1.0
    assert P.max() <= 15, "fp8e4 exactness bound exceeded"

    dinv_pad = np.zeros(NPAD, np.float32)
    dinv_pad[pself] = dinv

    # fp8 scale calibration: host fp32 forward to find per-layer max|x_l|
    import scipy.sparse as sp

    A = sp.csr_matrix(
        (np.ones(len(src) + N_NODES, np.float32),
         (np.concatenate([dst, np.arange(N_NODES)]),
          np.concatenate([src, np.arange(N_NODES)]))),
        shape=(N_NODES, N_NODES),
    )
    g = dinv[:, None] * x
    scales = []
    for li in range(8):
        W = np.asarray(inputs[f"W{li}"], np.float32)
        b = np.asarray(inputs[f"b{li}"], np.float32)
        xx = g @ W
        mx = max(float(np.abs(xx).max()), 1e-20)
        scales.append(float(2.0 ** np.floor(np.log2(224.0 / mx))))
        agg = A @ xx
        h = np.maximum(dinv[:, None] ** 2 * agg + dinv[:, None] * b, 0.0)
        g = h

    # initial g0 = dinv * x, feature-major, padded
    g0 = dinv[:, None] * x  # [N, 128]
    g0t = np.zeros((128, NPAD), np.float32)
    g0t[:, pself] = g0.T
    g0t = g0t.astype(F16)

    # weights (scale folded in): [128, ftiles, dout], zero-padded contraction
    w_np = []
    for li in range(8):
        W = np.asarray(inputs[f"W{li}"], np.float32) * scales[li]
        ft = FTILES[li]
        Wp = np.zeros((ft * 128, DIMS[li + 1]), np.float32)
        Wp[: W.shape[0]] = W
        w_np.append(np.ascontiguousarray(Wp.reshape(ft, 128, DIMS[li + 1]).transpose(1, 0, 2)).astype(F16))
    bb_np = [
        np.broadcast_to(np.asarray(inputs[f"b{li}"], np.float32), (128, DIMS[li + 1])).copy()
        for li in range(8)
    ]
    wr = np.asarray(inputs["Wr"], np.float32)  # [32, 1]
    wr_np = np.broadcast_to(wr[:, 0], (128, DIMS[-1])).copy()
    br_np = np.full((128, 1), np.asarray(inputs["br"], np.float32).reshape(()), np.float32)

    in_maps = []
    for k in range(N_CORES):
        rows = slice(k * SHARD_PAD, (k + 1) * SHARD_PAD)
        # PT swizzle: pt[p, jt, w, c] = P[k*1280 + jt*128 + c, w*128 + p]
        S = P[rows].reshape(JTILES, 128, WTILES, 128)  # [jt, c, w, p]
        pt = np.ascontiguousarray(S.transpose(3, 0, 2, 1)).astype(FP8)
        dj = dinv_pad[rows].reshape(JTILES, 128).T.copy()  # [128, JTILES]
        dinv2s = np.stack([dj * dj / scales[li] for li in range(7)], axis=1)
        m = {
            "pt_in": pt,
            "g0t_in": g0t,
            "dinv1_in": dj,
            "dinv2s_in": np.ascontiguousarray(dinv2s, np.float32),
            "dinv17_in": (dj / scales[7]).astype(np.float32),
            "wr_in": wr_np,
            "br_in": br_np,
        }
        for li in range(8):
            m[f"w{li}_in"] = w_np[li]
            m[f"bb{li}_in"] = bb_np[li]
        in_maps.append(m)
    return in_maps


def kernel(**inputs):
    nc = _get_nc()
    in_maps = _prepare_inputs(inputs)
    res = bass_utils.run_bass_kernel_spmd(nc, in_maps, core_ids=list(range(N_CORES)))
    out = np.empty((N_NODES, D_OUT), np.float32)
    for k in range(N_CORES):
        shard = res.results[k]["out"].reshape(SHARD_PAD, D_OUT)
        out[k * SHARD : (k + 1) * SHARD] = shard[:SHARD]
    return out
